# revision 1
# baseline (speedup 1.0000x reference)
"""3-layer GAT on trn2, 8 NeuronCores (SPMD).

Strategy:
- Nodes are permuted and dealt to 8 cores (snake order by in-degree);
  each core owns a contiguous SHARD of table rows and the dst-blocks
  (<=128 dst nodes each) made from them.
- Per layer: each core transforms its shard (feat|el|er = h @ [W|Wl|Wr],
  PE matmuls on DMA-transposed h tiles) into a staging tile, one DMA to
  the DRAM bounce, AllGather of the 512B-row fp16 table (Shared output,
  one table per layer), then an edge phase over 4 src-index ranges
  (dma_gather is int16-indexed, so the 100352-row table is addressed via
  4 base slices; pad indices point at always-zero dummy rows).
- Edge phase, per 1024-edge gather call (round-robin over 4 SWDGE
  queues): dma_gather of [feat|el] rows by src; a K=1 ones-matmul
  broadcasts the call's dst_local row into PSUM; is_equal builds the
  transposed one-hot OT and edge-major one-hot OE; per 128-edge chunk
  two PE matmuls deliver el+er into PSUM (identity @ el_cols + OT @
  er_block); one fused DVE op computes leaky-relu, ACT exp writes w
  straight into V[:,:,128:132]; DVE scales feat into V[:,:,0:128];
  per chunk the main PE matmul OneHot^T @ V accumulates [dst, 132] in
  PSUM; at each (block, pass) boundary PSUM is added into a per-layer
  SBUF accumulator.
- Epilogue in groups of 14 blocks: batched reciprocal/residual/ELU (or
  head-mean on the last layer) with one DMA per group for loads/stores.
"""
import numpy as np

import concourse.bacc as bacc
import concourse.bass as bass
import concourse.mybir as mybir
import concourse.tile as tile
from concourse.bass_utils import run_bass_kernel_spmd

P = 128
NCORES = 8
HEADS = 4
F = 32
D = 128            # feature dim (all layers)
TW = 256           # table row elems fp16 (512B)
RANGE = 32768      # rows per gather base slice (int16 idx limit)
CALL_CH = 8        # chunks per gather call (1024 idxs)
EPG = 14           # epilogue blocks per group (98 = 7*14)
fp16 = mybir.dt.float16
fp32 = mybir.dt.float32
AF = mybir.ActivationFunctionType
OP = mybir.AluOpType

_CACHE = {}
LAST_RESULTS = None
PREPARE_ONLY = False   # bench hook: stash (nc, in_maps) and skip HW run
LAST_PREP = None
ABLATE = ""            # bench hook: {noedge, gatheronly, noer, nocoll}
QUEUES = 4             # SWDGE queues for dma_gather round-robin
SINGLE_PACKET = True   # dma_gather single_packet flag
GATHER_BYTES = 512     # bytes gathered per edge (512 full row)
DMA_SCRATCH = 16384    # SWDGE descriptor ring bytes (16B/desc, per queue)


# ----------------------------------------------------------------- host side
def _preprocess(src, dst, n_nodes):
    """Build node permutation, per-core schedules and index tiles."""
    E = src.shape[0]
    shard = int(np.ceil(n_nodes / (NCORES * P))) * P          # 12544
    ntot = NCORES * shard
    deg = np.bincount(dst, minlength=n_nodes)
    order = np.argsort(-deg, kind="stable")                    # desc degree
    # snake deal to cores
    core_of_pos = np.tile(np.concatenate([np.arange(NCORES),
                                          np.arange(NCORES)[::-1]]),
                          (n_nodes + 2 * NCORES - 1) // (2 * NCORES))[:n_nodes]
    node_core = np.empty(n_nodes, np.int64)
    node_core[order] = core_of_pos
    # position within core (by deal order -> desc degree within core)
    perm = np.empty(n_nodes, np.int64)                         # node -> table row
    core_nodes = []
    for c in range(NCORES):
        nodes_c = order[node_core[order] == c]
        core_nodes.append(nodes_c)
        perm[nodes_c] = c * shard + np.arange(len(nodes_c))
    row_of_node = perm
    # per-core blocks: consecutive 128 rows of the shard
    gmax = shard // P                                          # 98 blocks hold real nodes
    # pad gather targets: one always-dummy (zero) row per range
    pad_local = np.zeros(4, np.int16)
    for r in range(4):
        found = False
        for c in range(NCORES):
            g = c * shard + len(core_nodes[c])                 # first dummy row
            ge = (c + 1) * shard - 1
            lo, hi = r * RANGE, (r + 1) * RANGE - 1
            g0 = max(g, lo)
            if g0 <= min(ge, hi):
                pad_local[r] = np.int16(g0 - r * RANGE)
                found = True
                break
        assert found, f"no dummy row in range {r}"
    # group edges: core/block of dst, range of src
    e_core = node_core[dst]
    e_blk = (row_of_node[dst] % shard) // P
    e_rng = row_of_node[src] // RANGE
    e_idx = (row_of_node[src] % RANGE).astype(np.int16)
    e_dloc = (row_of_node[dst] % P).astype(np.int16)

    # counts[c, b, r]
    counts = np.zeros((NCORES, gmax, 4), np.int64)
    np.add.at(counts, (e_core, e_blk, e_rng), 1)
    # chunk table l[b, r] = max over cores (blocks are degree-sorted so the
    # per-position max is tight); at least 1 so every block gets a pass-0 copy
    l = np.maximum(np.ceil(counts / P).astype(np.int64).max(axis=0), 1)  # [g, 4]
    G = gmax
    chunks_r = l.sum(axis=0)                                   # per pass
    chmax = int(chunks_r.max())
    ncalls_r = [int(np.ceil(c / CALL_CH)) for c in chunks_r]
    callmax = max(ncalls_r)

    # per-core streams
    # slot start of (b, r) within pass r: prefix over b of l[:, r]
    starts = np.zeros((G, 4), np.int64)
    starts[1:] = np.cumsum(l[:-1], axis=0)

    CW = CALL_CH * P // 16
    idx_tiles = np.zeros((NCORES, 4, P, callmax * CW), np.int16)
    dcol = np.full((NCORES, 4, P, chmax), 240, np.float16)
    drow = np.full((NCORES, 4, 1, chmax * P), 240, np.float16)

    eo = np.lexsort((e_rng, e_blk, e_core))                    # group edges
    es, eb, er_, ei, ed = (x[eo] for x in (e_core, e_blk, e_rng, e_idx, e_dloc))
    # offsets within group via cumcount
    grp = es * (gmax * 4) + eb * 4 + er_
    sort_count = np.bincount(grp, minlength=NCORES * gmax * 4)
    within = np.arange(len(eo)) - np.repeat(
        np.concatenate([[0], np.cumsum(sort_count)[:-1]]), sort_count)

    slot = starts[eb, er_] * P + within                        # edge slot in pass
    # fill idx / dloc arrays
    NI = CALL_CH * P
    for c in range(NCORES):
        m = es == c
        s, r, iv, dv = slot[m], er_[m], ei[m], ed[m]
        for rr in range(4):
            mm = r == rr
            ss = s[mm]
            flat_i = np.full(chunks_r[rr] * P, pad_local[rr], np.int16)
            flat_d = np.full(chunks_r[rr] * P, 240, np.int16)
            flat_i[ss] = iv[mm]
            flat_d[ss] = dv[mm]
            drow[c, rr, 0, :chunks_r[rr] * P] = flat_d.astype(np.float16)
            dcol[c, rr, :, :chunks_r[rr]] = (
                flat_d.reshape(chunks_r[rr], P).T.astype(np.float16))
            # idx tiles per call: i-th idx of call k at [16a+(i%16), i//16]
            for k in range(ncalls_r[rr]):
                seg = flat_i[k * NI:(k + 1) * NI]
                n = len(seg)
                if n < NI:
                    seg = np.concatenate(
                        [seg, np.full(NI - n, pad_local[rr], np.int16)])
                idx_tiles[c, rr, :, k * CW:(k + 1) * CW] = np.tile(
                    seg.reshape(NI // 16, 16).T, (8, 1))

    sched = dict(shard=shard, ntot=ntot, G=G, l=l, chunks_r=chunks_r,
                 ncalls_r=ncalls_r, callmax=callmax, chmax=chmax,
                 starts=starts)
    data = dict(idx_tiles=idx_tiles, dcol=dcol, drow=drow)
    return perm, core_nodes, sched, data


# ------------------------------------------------------------- device program
def _build(sched):
    shard, G = sched["shard"], sched["G"]
    l, chunks_r, ncalls_r = sched["l"], sched["chunks_r"], sched["ncalls_r"]
    callmax, chmax = sched["callmax"], sched["chmax"]
    GR = G * P                                                 # rows incl dummy blocks
    NT = sched["ntot"]
    CW = CALL_CH * P // 16

    nc = bacc.Bacc("TRN2", target_bir_lowering=False, debug=False,
                   num_devices=NCORES, num_swdge_queues=QUEUES,
                   dynamic_dma_scratch_size=DMA_SCRATCH)
    feats = nc.dram_tensor("feats", [GR, D], fp16, kind="ExternalInput")
    wcat = nc.dram_tensor("wcat", [3, D, 136], fp16, kind="ExternalInput")
    idx_t = nc.dram_tensor("idx_t", [4, P, callmax * CW], mybir.dt.int16,
                           kind="ExternalInput")
    dcol_t = nc.dram_tensor("dcol_t", [4, P, chmax], fp16, kind="ExternalInput")
    drow_t = nc.dram_tensor("drow_t", [4, 1, chmax * P], fp16,
                            kind="ExternalInput")
    out_sh = nc.dram_tensor("out_sh", [GR, F], fp32, kind="ExternalOutput")

    # prefix sums for chunk -> block mapping per pass
    starts = sched["starts"]
    qctr = [0]

    with tile.TileContext(nc) as tc:
        with (
            tc.tile_pool(name="const", bufs=1) as cp,
            tc.tile_pool(name="dram", bufs=1, space="DRAM") as dramp,
            tc.tile_pool(name="io", bufs=8) as iop,
            tc.tile_pool(name="slab", bufs=2) as slabp,
            tc.tile_pool(name="gath", bufs=7) as gp,
            tc.tile_pool(name="oh", bufs=5) as ohp,
            tc.tile_pool(name="sm", bufs=4) as smp,
            tc.tile_pool(name="acc", bufs=1) as accp,
            tc.tile_pool(name="epi", bufs=1) as epip,
            tc.tile_pool(name="ps", bufs=4, space="PSUM") as psp,
            tc.tile_pool(name="psb", bufs=1, space="PSUM") as psbp,
            tc.tile_pool(name="pse", bufs=2, space="PSUM") as psep,
        ):
            # DRAM scratch
            h1 = dramp.tile([GR, D], fp16)
            h2 = dramp.tile([GR, D], fp16)
            bounce = dramp.tile([GR, TW], fp16)
            tables = [dramp.tile([4 * RANGE, TW], fp16, addr_space="Shared",
                                 name=f"table{L}") for L in range(3)]

            # constants
            iota = cp.tile([P, P], fp16)
            nc.gpsimd.iota(iota[:], pattern=[[1, P]], base=0,
                           channel_multiplier=0,
                           allow_small_or_imprecise_dtypes=True)
            iota8 = cp.tile([P, CALL_CH, P], fp16)
            nc.gpsimd.iota(iota8[:], pattern=[[0, CALL_CH], [1, P]], base=0,
                           channel_multiplier=0,
                           allow_small_or_imprecise_dtypes=True)
            chiota = cp.tile([P, 1], fp32)
            nc.gpsimd.iota(chiota[:], pattern=[[0, 1]], base=0,
                           channel_multiplier=1,
                           allow_small_or_imprecise_dtypes=True)
            ident = cp.tile([P, P], fp16)
            nc.vector.tensor_scalar(ident[:], iota[:], chiota[:], None,
                                    op0=OP.is_equal)
            ones_row = cp.tile([1, P], fp16)
            nc.vector.memset(ones_row[:], 1.0)

            wcs = []
            for L in range(3):
                wc = cp.tile([D, 136], fp16, tag="wc")
                nc.sync.dma_start(wc[:], wcat[L])
                wcs.append(wc)

            h_of = {0: feats[:], 1: h1[:], 2: h2[:]}
            stf_of = {}

            def emit_transform(L, t0, t1):
                """stf[:, t, 0:136] = hT.T @ wcat for blocks [t0, t1)."""
                if L not in stf_of:
                    stf_of[L] = smp.tile([P, G, 136], fp16, tag="stf",
                                         bufs=1, name=f"stf{L}")
                stf_all = stf_of[L]
                for t in range(t0, t1):
                    hT = smp.tile([P, P], fp16, tag="hT")
                    nc.sync.dma_start(hT[:], h_of[L][t * P:(t + 1) * P, :],
                                      transpose=True)
                    ptf = psp.tile([P, 136], fp32, space="PSUM", tag="pm")
                    nc.tensor.matmul(ptf[:], hT[:], wcs[L][:],
                                     start=True, stop=True)
                    nc.vector.tensor_copy(stf_all[:, t, :], ptf[:])

            emit_transform(0, 0, G)

            for L in range(3):
                table = tables[L]
                nc.sync.dma_start(
                    bounce[0:GR, 0:136].rearrange("(b p) f -> p b f", p=P),
                    stf_of[L][:])

                # ---- allgather the padded-row table
                if ABLATE != "nocoll":
                    nc.gpsimd.collective_compute(
                        "AllGather", OP.bypass,
                        replica_groups=[list(range(NCORES))],
                        ins=[bounce[0:shard, :].opt()],
                        outs=[table[0:NT, :].opt()])

                # er per block, resident: er_all[p, b, h] = bounce[b*128+p, 132+h]
                er_all = smp.tile([P, G, 4], fp16, tag="er_all")
                nc.sync.dma_start(
                    er_all[:],
                    bounce[0:GR, 132:136].rearrange("(b p) h -> p b h", p=P))

                # per-layer accumulator [P, G, 132]
                acc = accp.tile([P, G, 132], fp32, tag="acc",
                                name=f"acc{L}")
                first_pass = [True] * G

                def emit_epilogue(L, acc, g0, g1):
                    ng = g1 - g0
                    rec = epip.tile([P, EPG, 4], fp32, tag="rec")
                    nc.vector.reciprocal(rec[:, 0:ng, :],
                                         acc[:, g0:g1, 128:132])
                    av = epip.tile([P, EPG, D], fp32, tag="av")
                    nc.vector.tensor_tensor(
                        out=av[:, 0:ng, :].rearrange(
                            "p b (h f) -> p b h f", f=F),
                        in0=acc[:, g0:g1, 0:128].rearrange(
                            "p b (h f) -> p b h f", f=F),
                        in1=rec[:, 0:ng, :].unsqueeze(3)
                            .to_broadcast([P, ng, HEADS, F]),
                        op=OP.mult)
                    if L >= 1:  # residual
                        hres = epip.tile([P, EPG, D], fp16, tag="hres")
                        nc.sync.dma_start(
                            hres[:, 0:ng, :],
                            h_of[L][g0 * P:g1 * P, :].rearrange(
                                "(b p) d -> p b d", p=P))
                        nc.vector.tensor_tensor(
                            out=av[:, 0:ng, :], in0=av[:, 0:ng, :],
                            in1=hres[:, 0:ng, :], op=OP.add)
                    if L < 2:
                        # ELU: relu(x) + exp(min(x,0)) - 1
                        relu = epip.tile([P, EPG * D], fp16, tag="relu")
                        flat = av[:, 0:ng, :].rearrange("p b d -> p (b d)")
                        nc.vector.tensor_scalar_max(relu[:, 0:ng * D],
                                                    flat, 0.0)
                        nc.vector.tensor_scalar_min(flat, flat, 0.0)
                        ex = epip.tile([P, EPG * D], fp16, tag="ex")
                        nc.scalar.activation(ex[:, 0:ng * D], flat, AF.Exp)
                        hnext = epip.tile([P, EPG, D], fp16, tag="hnext")
                        nc.vector.scalar_tensor_tensor(
                            hnext[:, 0:ng, :].rearrange("p b d -> p (b d)"),
                            ex[:, 0:ng * D], -1.0, relu[:, 0:ng * D],
                            op0=OP.add, op1=OP.add)
                        nc.sync.dma_start(
                            h_of[L + 1][g0 * P:g1 * P, :].rearrange(
                                "(b p) d -> p b d", p=P),
                            hnext[:, 0:ng, :])
                        emit_transform(L + 1, g0, g1)
                    else:
                        # mean over heads
                        o32 = epip.tile([P, EPG, F], fp32, tag="o32")
                        avh = av[:, 0:ng, :].rearrange(
                            "p b (h f) -> p b h f", f=F)
                        nc.vector.tensor_tensor(out=o32[:, 0:ng, :],
                                                in0=avh[:, :, 0, :],
                                                in1=avh[:, :, 1, :],
                                                op=OP.add)
                        nc.vector.tensor_tensor(out=o32[:, 0:ng, :],
                                                in0=o32[:, 0:ng, :],
                                                in1=avh[:, :, 2, :],
                                                op=OP.add)
                        nc.vector.tensor_tensor(out=o32[:, 0:ng, :],
                                                in0=o32[:, 0:ng, :],
                                                in1=avh[:, :, 3, :],
                                                op=OP.add)
                        nc.vector.tensor_scalar_mul(o32[:, 0:ng, :],
                                                    o32[:, 0:ng, :], 0.25)
                        nc.sync.dma_start(
                            out_sh[g0 * P:g1 * P, :].rearrange(
                                "(b p) f -> p b f", p=P),
                            o32[:, 0:ng, :])

                ep_ptr = [0]

                for r in range(4) if ABLATE != "noedge" else []:
                    dcol_s = slabp.tile([P, chmax], fp16, tag="dcol")
                    nc.sync.dma_start(dcol_s[:], dcol_t[r])
                    idxs = slabp.tile([P, callmax * CW], mybir.dt.int16,
                                      tag="idx")
                    nc.sync.dma_start(idxs[:, 0:ncalls_r[r] * CW],
                                      idx_t[r, :, 0:ncalls_r[r] * CW])

                    nch = int(chunks_r[r])
                    # block segment boundaries in this pass
                    blk_of = np.repeat(np.arange(G), l[:, r])
                    pm_tiles = {}
                    for k in range(ncalls_r[r]):
                        c0 = k * CALL_CH
                        ncc = min(CALL_CH, nch - c0)
                        ni = ncc * P
                        Gt = gp.tile([P, CALL_CH, TW], fp16, tag="G")
                        nc.gpsimd.dma_gather(
                            Gt[:, 0:ncc, :],
                            table[r * RANGE:(r + 1) * RANGE, :],
                            idxs[:, k * CW:k * CW + ni // 16],
                            num_idxs=ni, num_idxs_reg=ni,
                            elem_size=TW,
                            queue_num=qctr[0] % QUEUES,
                            single_packet=SINGLE_PACKET)
                        qctr[0] += 1
                        if ABLATE == "gatheronly":
                            continue
                        drow_c = iop.tile([1, CALL_CH * P], fp16, tag="drow")
                        nc.sync.dma_start(drow_c[:, 0:ni],
                                          drow_t[r, :, c0 * P:c0 * P + ni])
                        # bcast dst_local row into psum -> transposed one-hot
                        OE = ohp.tile([P, CALL_CH, P], fp16, tag="OE")
                        nc.vector.tensor_tensor(
                            out=OE[:, 0:ncc, :],
                            in0=iota8[:, 0:ncc, :],
                            in1=dcol_s[:, c0:c0 + ncc].unsqueeze(2)
                                .to_broadcast([P, ncc, P]),
                            op=OP.is_equal)
                        if ABLATE != "noer":
                            pbc = psbp.tile([P, CALL_CH * P], fp32,
                                            space="PSUM", tag="pbc")
                            for hh in range(0, ni, 512):
                                hw = min(512, ni - hh)
                                nc.tensor.matmul(
                                    pbc[:, hh:hh + hw], ones_row[:],
                                    drow_c[:, hh:hh + hw],
                                    start=True, stop=True)
                            OT = ohp.tile([P, CALL_CH, P], fp16, tag="OT")
                            nc.vector.tensor_scalar(
                                OT[:, 0:ncc, :],
                                pbc[:, 0:ni].rearrange("p (c e) -> p c e", e=P),
                                chiota[:], None, op0=OP.is_equal)
                            # e = el + er into PSUM: identity@el + OT@er
                            erp = psep.tile([P, CALL_CH * 4], fp32,
                                            space="PSUM", tag="er")
                            for c in range(ncc):
                                b = int(blk_of[c0 + c])
                                nc.tensor.matmul(erp[:, c * 4:(c + 1) * 4],
                                                 ident[:],
                                                 Gt[:, c, 128:132],
                                                 start=True, stop=False)
                                nc.tensor.matmul(erp[:, c * 4:(c + 1) * 4],
                                                 OT[:, c, :], er_all[:, b, :],
                                                 start=False, stop=True)
                            esrc = erp[:, 0:ncc * 4]
                        else:
                            e32 = smp.tile([P, CALL_CH * 4], fp32, tag="e32")
                            nc.vector.tensor_copy(
                                e32[:, 0:ncc * 4]
                                    .rearrange("p (c h) -> p c h", h=4),
                                Gt[:, 0:ncc, 128:132])
                            esrc = e32[:, 0:ncc * 4]
                        # leaky-relu: lr = max(0.2*e, e); ACT scales, DVE maxes
                        e02 = smp.tile([P, CALL_CH * 4], fp32, tag="e02")
                        nc.scalar.activation(e02[:, 0:ncc * 4], esrc,
                                             AF.Copy, scale=0.2)
                        lr = smp.tile([P, CALL_CH * 4], fp32, tag="lr")
                        nc.vector.tensor_tensor(
                            out=lr[:, 0:ncc * 4], in0=e02[:, 0:ncc * 4],
                            in1=esrc, op=OP.max)
                        V = gp.tile([P, CALL_CH, 132], fp16, tag="V")
                        nc.scalar.activation(
                            V[:, 0:ncc, 128:132],
                            lr[:, 0:ncc * 4].rearrange("p (c h) -> p c h", h=4),
                            AF.Exp)
                        nc.vector.tensor_tensor(
                            out=V[:, 0:ncc, 0:128]
                                .rearrange("p c (h f) -> p c h f", f=F),
                            in0=Gt[:, 0:ncc, 0:128]
                                .rearrange("p c (h f) -> p c h f", f=F),
                            in1=V[:, 0:ncc, 128:132]
                                .unsqueeze(3).to_broadcast([P, ncc, 4, F]),
                            op=OP.mult)
                        # main matmuls, accumulate per block
                        for c in range(ncc):
                            gc = c0 + c
                            b = int(blk_of[gc])
                            sb_, lb = int(starts[b, r]), int(l[b, r])
                            if b not in pm_tiles:
                                pm_tiles[b] = psp.tile(
                                    [P, 136], fp32, space="PSUM", tag="pm",
                                    name=f"pm{L}_{r}_{b}")
                            nc.tensor.matmul(pm_tiles[b][:, 0:132],
                                             OE[:, c, :], V[:, c, :],
                                             start=(gc == sb_),
                                             stop=(gc == sb_ + lb - 1))
                            if gc == sb_ + lb - 1:
                                if first_pass[b]:
                                    nc.vector.tensor_copy(
                                        acc[:, b, :], pm_tiles[b][:, 0:132])
                                    first_pass[b] = False
                                else:
                                    nc.vector.tensor_tensor(
                                        out=acc[:, b, :], in0=acc[:, b, :],
                                        in1=pm_tiles[b][:, 0:132], op=OP.add)
                                del pm_tiles[b]

                if ABLATE in ("noedge", "gatheronly"):
                    nc.vector.memset(acc[:], 1.0)

                # flush remaining epilogue groups (ablate paths / leftovers)
                while ep_ptr[0] < G:
                    emit_epilogue(L, acc, ep_ptr[0], min(ep_ptr[0] + EPG, G))
                    ep_ptr[0] += EPG
    nc.compile()
    return nc


def kernel(**inputs):
    feats_f32 = np.asarray(inputs["features"], np.float32)
    src = np.asarray(inputs["src"]).astype(np.int64)
    dst = np.asarray(inputs["dst"]).astype(np.int64)
    n_nodes = feats_f32.shape[0]

    perm, core_nodes, sched, data = _preprocess(src, dst, n_nodes)
    shard, G = sched["shard"], sched["G"]

    # weights: Wcat[L] = [W | Wl | Wr] with Wl = sum_f W[:,h,f]*al[h,f]
    wcat = np.zeros((3, D, 136), np.float16)
    for L, (wn, an, bn) in enumerate([("W0", "al0", "ar0"),
                                      ("W1", "al1", "ar1"),
                                      ("W2", "al2", "ar2")]):
        W = np.asarray(inputs[wn], np.float32)
        al = np.asarray(inputs[an], np.float32)
        ar = np.asarray(inputs[bn], np.float32)
        Wh = W.reshape(D, HEADS, F)
        wcat[L, :, 0:128] = W.astype(np.float16)
        wcat[L, :, 128:132] = np.einsum("dhf,hf->dh", Wh, al).astype(np.float16)
        wcat[L, :, 132:136] = np.einsum("dhf,hf->dh", Wh, ar).astype(np.float16)

    key = (n_nodes, src.shape[0])
    if key not in _CACHE:
        _CACHE[key] = _build(sched)
    nc = _CACHE[key]

    in_maps = []
    for c in range(NCORES):
        f16 = np.zeros((G * P, D), np.float16)
        nodes_c = core_nodes[c]
        f16[:len(nodes_c)] = feats_f32[nodes_c].astype(np.float16)
        in_maps.append({
            "feats": f16,
            "wcat": wcat,
            "idx_t": data["idx_tiles"][c],
            "dcol_t": data["dcol"][c],
            "drow_t": data["drow"][c],
        })

    global LAST_PREP
    if PREPARE_ONLY:
        LAST_PREP = (nc, in_maps, core_nodes)
        return np.zeros((n_nodes, F), np.float32)

    import os
    trace = bool(int(os.environ.get("TRN_KERNEL_TRACE", "0")))
    res = run_bass_kernel_spmd(nc, in_maps, core_ids=list(range(NCORES)),
                               trace=trace)
    global LAST_RESULTS
    LAST_RESULTS = res
    out = np.zeros((n_nodes, F), np.float32)
    for c in range(NCORES):
        nodes_c = core_nodes[c]
        out[nodes_c] = res.results[c]["out_sh"][:len(nodes_c)]
    return out



# revision 22
# speedup vs baseline: 1.1531x; 1.1531x over previous
"""3-layer GAT on trn2, 8 NeuronCores (SPMD), group-pipelined.

Strategy:
- Nodes are snake-dealt by in-degree to 8 cores (12500 real rows each,
  padded to 12544 = 98 blocks of 128).  Table rows are GROUP-major:
  7 groups x (8 cores x 1792 rows), so each group's AllGather writes a
  contiguous 14336-row slice and collectives pipeline into the edge
  phase of the same / previous layer.
- Per layer, per group of 14 dst blocks: 4 range passes of dma_gather
  calls (<=1024 edges, int16 idx per 32768-row base slice; 512B rows
  [feat16|el|er]), then epilogue (softmax denom, residual, ELU or head
  mean), next-layer transform (PE transpose + matmul, no DMA
  transpose), bounce write and this group's AllGather.
- Per 128-edge chunk: OE one-hot from iota vs dst_local (DVE
  tensor_scalar), OT = PE-transpose(OE) + ACT copy; e = el + er via two
  PE matmuls into PSUM; ACT Lrelu(alpha=.2) + ACT Exp -> w; DVE scales
  feat into V; OE^T @ V accumulates into the block's PSUM tile across
  all 4 ranges; one copy into the group accumulator at block end.
"""
import numpy as np

import concourse.bacc as bacc
import concourse.bass as bass
import concourse.mybir as mybir
import concourse.tile as tile
from concourse.bass_utils import run_bass_kernel_spmd

P = 128
NCORES = 8
HEADS = 4
F = 32
D = 128            # feature dim (all layers)
TW = 256           # table row elems fp16 (512B)
CALL_CH = 8        # chunks per gather call (1024 idxs)
GS = 14            # blocks per group
NG = 7             # groups (G = 98 blocks)
G = NG * GS
SLICE = GS * P     # 1792 rows per (core, group)
TGW = NCORES * SLICE  # 14336 table rows per group = one gather base slice
NR = NG            # idx ranges == table groups
fp16 = mybir.dt.float16
fp32 = mybir.dt.float32
AF = mybir.ActivationFunctionType
OP = mybir.AluOpType

_CACHE = {}
LAST_RESULTS = None
PREPARE_ONLY = False   # bench hook: stash (nc, in_maps) and skip HW run
LAST_PREP = None
QUEUES = 4             # SWDGE queues for dma_gather round-robin
DMA_SCRATCH = 32768    # SWDGE descriptor ring bytes (2 calls deep)
OE_TS = False          # per-chunk tensor_scalar OE (SLOW on hw)


# ----------------------------------------------------------------- host side
def _preprocess(src, dst, n_nodes):
    """Node permutation, group-major table rows, per-core idx tiles."""
    percore = n_nodes // NCORES
    assert percore * NCORES == n_nodes and percore <= NG * SLICE
    shard = NG * SLICE                                         # 12544
    deg = np.bincount(dst, minlength=n_nodes)
    order = np.argsort(-deg, kind="stable")                    # desc degree
    core_of_pos = np.tile(np.concatenate([np.arange(NCORES),
                                          np.arange(NCORES)[::-1]]),
                          (n_nodes + 2 * NCORES - 1) // (2 * NCORES))[:n_nodes]
    node_core = np.empty(n_nodes, np.int64)
    node_core[order] = core_of_pos
    pos = np.empty(n_nodes, np.int64)
    core_nodes = []
    for c in range(NCORES):
        nodes_c = order[node_core[order] == c]
        assert len(nodes_c) == percore
        core_nodes.append(nodes_c)
        pos[nodes_c] = np.arange(percore)
    # group-major table row of each node
    trow = (pos // SLICE) * TGW + node_core * SLICE + (pos % SLICE)

    e_core = node_core[dst]
    p_dst = pos[dst]
    e_blk = p_dst // P
    e_dloc = (p_dst % P).astype(np.int16)
    e_rng = trow[src] // TGW
    e_idx = (trow[src] % TGW).astype(np.int16)

    counts = np.zeros((NCORES, G, NR), np.int64)
    np.add.at(counts, (e_core, e_blk, e_rng), 1)
    l = np.maximum(np.ceil(counts / P).astype(np.int64).max(axis=0), 1)
    chunks_r = l.sum(axis=0)                                   # [NR]
    chmax = int(chunks_r.max())
    starts = np.zeros((G, NR), np.int64)
    starts[1:] = np.cumsum(l[:-1], axis=0)

    # per-(group, range) gather-call segmentation (same for all cores)
    group_calls = [[] for _ in range(NR)]     # [r][g] -> [(c0, ncc, col)]
    cols_r = [0] * NR
    for r in range(NR):
        col = 0
        for g in range(NG):
            cg0 = int(starts[g * GS, r])
            cgn = int(l[g * GS:(g + 1) * GS, r].sum())
            calls = []
            for k in range((cgn + CALL_CH - 1) // CALL_CH):
                ncc = min(CALL_CH, cgn - k * CALL_CH)
                calls.append((cg0 + k * CALL_CH, ncc, col))
                col += ncc * P // 16
            group_calls[r].append(calls)
        cols_r[r] = col
    colsmax = max(cols_r)

    idx_tiles = np.zeros((NCORES, NR, P, colsmax), np.int16)
    dcol = np.full((NCORES, NR, P, chmax), 240, np.float16)

    eo = np.lexsort((e_rng, e_blk, e_core))
    es, eb, er_, ei, ed = (x[eo] for x in (e_core, e_blk, e_rng, e_idx, e_dloc))
    grp = es * (G * NR) + eb * NR + er_
    sort_count = np.bincount(grp, minlength=NCORES * G * NR)
    within = np.arange(len(eo)) - np.repeat(
        np.concatenate([[0], np.cumsum(sort_count)[:-1]]), sort_count)
    slot = starts[eb, er_] * P + within

    for c in range(NCORES):
        m = es == c
        s, r, iv, dv = slot[m], er_[m], ei[m], ed[m]
        for rr in range(NR):
            mm = r == rr
            nch = int(chunks_r[rr])
            flat_i = np.zeros(nch * P, np.int16)          # pad -> row 0 (real)
            flat_d = np.full(nch * P, 240, np.int16)      # pad dst -> no one-hot
            flat_i[s[mm]] = iv[mm]
            flat_d[s[mm]] = dv[mm]
            dcol[c, rr, :, :nch] = flat_d.reshape(nch, P).T.astype(np.float16)
            for calls in group_calls[rr]:
                for (c0, ncc, col) in calls:
                    ni = ncc * P
                    seg = flat_i[c0 * P:c0 * P + ni]
                    idx_tiles[c, rr, :, col:col + ni // 16] = np.tile(
                        seg.reshape(ni // 16, 16).T, (8, 1))

    sched = dict(shard=shard, percore=percore, l=l, starts=starts,
                 chunks_r=chunks_r, chmax=chmax, colsmax=colsmax,
                 group_calls=group_calls)
    data = dict(idx_tiles=idx_tiles, dcol=dcol)
    return core_nodes, sched, data


# ------------------------------------------------------------- device program
def _build(sched):
    l, starts = sched["l"], sched["starts"]
    chmax, colsmax = sched["chmax"], sched["colsmax"]
    group_calls = sched["group_calls"]
    percore = sched["percore"]
    GR = G * P                                                 # 12544
    NT = NCORES * GR                                           # 100352
    # dummy h rows (same on every core): local positions [percore, GR)
    dum_blk = (percore % SLICE) // P                           # block in g6
    dum_row = percore % P

    nc = bacc.Bacc("TRN2", target_bir_lowering=False, debug=False,
                   num_devices=NCORES, num_swdge_queues=QUEUES,
                   dynamic_dma_scratch_size=DMA_SCRATCH)
    feats = nc.dram_tensor("feats", [GR, D], fp16, kind="ExternalInput")
    wcat = nc.dram_tensor("wcat", [3, D, 136], fp16, kind="ExternalInput")
    idx_t = nc.dram_tensor("idx_t", [NR, P, colsmax], mybir.dt.int16,
                           kind="ExternalInput")
    dcol_t = nc.dram_tensor("dcol_t", [NR, P, chmax], fp16,
                            kind="ExternalInput")
    out_sh = nc.dram_tensor("out_sh", [GR, F], fp32, kind="ExternalOutput")

    qctr = [0]

    with tile.TileContext(nc) as tc:
        with (
            tc.tile_pool(name="const", bufs=1) as cp,
            tc.tile_pool(name="dram", bufs=1, space="DRAM") as dramp,
            tc.tile_pool(name="stf", bufs=2) as stfp,
            tc.tile_pool(name="hsb", bufs=2) as hsbp,
            tc.tile_pool(name="ht", bufs=3) as htp,
            tc.tile_pool(name="gath", bufs=11) as gp,
            tc.tile_pool(name="oh", bufs=6) as ohp,
            tc.tile_pool(name="ot", bufs=6) as otp,
            tc.tile_pool(name="sm", bufs=6) as smp,
            tc.tile_pool(name="acc", bufs=2) as accp,
            tc.tile_pool(name="epi", bufs=1) as epip,
            tc.tile_pool(name="ps", bufs=4, space="PSUM") as psp,
            tc.tile_pool(name="pst", bufs=2, space="PSUM") as pstp,
            tc.tile_pool(name="pse", bufs=2, space="PSUM") as psep,
        ):
            h1 = dramp.tile([GR, D], fp16)
            h2 = dramp.tile([GR, D], fp16)
            bounce = dramp.tile([GR, TW], fp16)
            tables = [[dramp.tile([TGW, TW], fp16, addr_space="Shared",
                                  name=f"table{L}_{g}") for g in range(NG)]
                      for L in range(3)]

            # constants
            iota = cp.tile([P, P], fp16)
            nc.gpsimd.iota(iota[:], pattern=[[1, P]], base=0,
                           channel_multiplier=0,
                           allow_small_or_imprecise_dtypes=True)
            iota8 = cp.tile([P, CALL_CH, P], fp16)
            nc.gpsimd.iota(iota8[:], pattern=[[0, CALL_CH], [1, P]], base=0,
                           channel_multiplier=0,
                           allow_small_or_imprecise_dtypes=True)
            chiota = cp.tile([P, 1], fp32)
            nc.gpsimd.iota(chiota[:], pattern=[[0, 1]], base=0,
                           channel_multiplier=1,
                           allow_small_or_imprecise_dtypes=True)
            ident = cp.tile([P, P], fp16)
            nc.vector.tensor_scalar(ident[:], iota[:], chiota[:], None,
                                    op0=OP.is_equal)
            wcs = []
            for L in range(3):
                wc = cp.tile([D, 136], fp16, tag=f"wc{L}", name=f"wc{L}")
                nc.sync.dma_start(wc[:], wcat[L])
                wcs.append(wc)

            # resident idx / dcol tiles (constant across layers)
            idxs_res, dcol_res = [], []
            for r in range(NR):
                it = cp.tile([P, colsmax], mybir.dt.int16, tag=f"idxr{r}",
                             name=f"idxr{r}")
                nc.sync.dma_start(it[:], idx_t[r])
                idxs_res.append(it)
                dc = cp.tile([P, chmax], fp16, tag=f"dcolr{r}",
                             name=f"dcolr{r}")
                nc.sync.dma_start(dc[:], dcol_t[r])
                dcol_res.append(dc)

            # per-layer er tables [P, G, 4] (small, persistent)
            er_all = [cp.tile([P, G, 4], fp16, tag=f"erall{L}",
                              name=f"er{L}") for L in range(3)]

            h_of = {0: feats[:], 1: h1[:], 2: h2[:]}

            def emit_transform(L, g, hsb):
                """Group g: stf = h @ [W|Wl|Wr]; bounce + AllGather slice."""
                stf_g = stfp.tile([P, GS, 136], fp16, tag="stf")
                for bi in range(GS):
                    pt = pstp.tile([P, P], fp16, space="PSUM", tag="tp")
                    nc.tensor.transpose(pt[:], hsb[:, bi, :], ident[:])
                    hT = htp.tile([P, P], fp16, tag="hT")
                    nc.scalar.activation(hT[:], pt[:], AF.Copy)
                    pf = psp.tile([P, 136], fp32, space="PSUM", tag="pm")
                    nc.tensor.matmul(pf[:], hT[:], wcs[L][:],
                                     start=True, stop=True)
                    nc.vector.tensor_copy(stf_g[:, bi, :], pf[:])
                nc.vector.tensor_copy(er_all[L][:, g * GS:(g + 1) * GS, :],
                                      stf_g[:, :, 132:136])
                nc.sync.dma_start(
                    bounce[g * SLICE:(g + 1) * SLICE, 0:136].rearrange(
                        "(b p) f -> p b f", p=P),
                    stf_g[:])
                nc.gpsimd.collective_compute(
                    "AllGather", OP.bypass,
                    replica_groups=[list(range(NCORES))],
                    ins=[bounce[g * SLICE:(g + 1) * SLICE, :].opt()],
                    outs=[tables[L][g][:].opt()])

            def emit_epilogue(L, g, acc_g):
                rec = epip.tile([P, GS, 4], fp32, tag="rec")
                nc.vector.tensor_scalar_add(rec[:], acc_g[:, :, 128:132],
                                            1e-6)
                nc.vector.reciprocal(rec[:], rec[:])
                av = epip.tile([P, GS, D], fp32, tag="av")
                nc.vector.tensor_tensor(
                    out=av[:].rearrange("p b (h f) -> p b h f", f=F),
                    in0=acc_g[:, :, 0:128].rearrange(
                        "p b (h f) -> p b h f", f=F),
                    in1=rec[:].unsqueeze(3).to_broadcast([P, GS, HEADS, F]),
                    op=OP.mult)
                if L >= 1:  # residual
                    hres = epip.tile([P, GS, D], fp16, tag="hres")
                    nc.sync.dma_start(
                        hres[:],
                        h_of[L][g * SLICE:(g + 1) * SLICE, :].rearrange(
                            "(b p) d -> p b d", p=P))
                    nc.vector.tensor_tensor(out=av[:], in0=av[:],
                                            in1=hres[:], op=OP.add)
                if L < 2:
                    # ELU: relu(x) + exp(min(x,0)) - 1
                    relu = epip.tile([P, GS * D], fp16, tag="relu")
                    flat = av[:].rearrange("p b d -> p (b d)")
                    nc.vector.tensor_scalar_max(relu[:], flat, 0.0)
                    nc.vector.tensor_scalar_min(flat, flat, 0.0)
                    ex = epip.tile([P, GS * D], fp16, tag="ex")
                    nc.scalar.activation(ex[:], flat, AF.Exp)
                    hnext = epip.tile([P, GS, D], fp16, tag="hnext")
                    nc.vector.scalar_tensor_tensor(
                        hnext[:].rearrange("p b d -> p (b d)"),
                        ex[:], -1.0, relu[:], op0=OP.add, op1=OP.add)
                    nc.sync.dma_start(
                        h_of[L + 1][g * SLICE:(g + 1) * SLICE, :].rearrange(
                            "(b p) d -> p b d", p=P),
                        hnext[:])
                    emit_transform(L + 1, g, hnext)
                else:
                    o32 = epip.tile([P, GS, F], fp32, tag="o32")
                    avh = av[:].rearrange("p b (h f) -> p b h f", f=F)
                    nc.vector.tensor_tensor(out=o32[:], in0=avh[:, :, 0, :],
                                            in1=avh[:, :, 1, :], op=OP.add)
                    nc.vector.tensor_tensor(out=o32[:], in0=o32[:],
                                            in1=avh[:, :, 2, :], op=OP.add)
                    nc.vector.tensor_tensor(out=o32[:], in0=o32[:],
                                            in1=avh[:, :, 3, :], op=OP.add)
                    nc.vector.tensor_scalar_mul(o32[:], o32[:], 0.25)
                    nc.sync.dma_start(
                        out_sh[g * SLICE:(g + 1) * SLICE, :].rearrange(
                            "(b p) f -> p b f", p=P),
                        o32[:])

            # ---- layer-0 prologue: per-group transform + collective
            for g in range(NG):
                hsb = hsbp.tile([P, GS, D], fp16, tag="hsb")
                nc.sync.dma_start(
                    hsb[:],
                    feats[g * SLICE:(g + 1) * SLICE, :].rearrange(
                        "(b p) d -> p b d", p=P))
                emit_transform(0, g, hsb)

            # ---- layers
            for L in range(3):
                blk_of = [np.repeat(np.arange(G), l[:, r])
                          for r in range(NR)]
                for g in range(NG):
                    acc_g = accp.tile([P, GS, 132], fp32, tag="acc")
                    for k in range(NR):
                        r = (g + k) % NR
                        pm_tiles = {}
                        for (c0, ncc, col) in group_calls[r][g]:
                            ni = ncc * P
                            Gt = gp.tile([P, CALL_CH, TW], fp16, tag="G")
                            nc.gpsimd.dma_gather(
                                Gt[:, 0:ncc, :],
                                tables[L][r][:],
                                idxs_res[r][:, col:col + ni // 16],
                                num_idxs=ni, num_idxs_reg=ni,
                                elem_size=TW,
                                queue_num=qctr[0] % QUEUES,
                                single_packet=True)
                            qctr[0] += 1
                            OE = ohp.tile([P, CALL_CH, P], fp16, tag="OE")
                            if OE_TS:
                                for c in range(ncc):
                                    nc.vector.tensor_scalar(
                                        OE[:, c, :], iota[:],
                                        dcol_res[r][:, c0 + c:c0 + c + 1],
                                        None, op0=OP.is_equal)
                            else:
                                nc.vector.tensor_tensor(
                                    out=OE[:, 0:ncc, :],
                                    in0=iota8[:, 0:ncc, :],
                                    in1=dcol_res[r][:, c0:c0 + ncc]
                                        .unsqueeze(2).to_broadcast([P, ncc, P]),
                                    op=OP.is_equal)
                            OTs = otp.tile([P, CALL_CH, P], fp16, tag="OT")
                            erp = psep.tile([P, CALL_CH * 4], fp32,
                                            space="PSUM", tag="er")
                            nc.tensor.matmul(
                                erp[:, 0:ncc * 4].rearrange(
                                    "p (c h) -> p c h", h=4),
                                ident[:], Gt[:, 0:ncc, 128:132],
                                start=True, stop=False)
                            for c in range(ncc):
                                b = int(blk_of[r][c0 + c])
                                pt = pstp.tile([P, P], fp16, space="PSUM",
                                               tag="tp")
                                nc.tensor.transpose(pt[:], OE[:, c, :],
                                                    ident[:])
                                nc.scalar.activation(OTs[:, c, :], pt[:],
                                                     AF.Copy)
                                nc.tensor.matmul(erp[:, c * 4:(c + 1) * 4],
                                                 OTs[:, c, :],
                                                 er_all[L][:, b, :],
                                                 start=False, stop=True)
                            e02 = smp.tile([P, CALL_CH * 4], fp32,
                                           tag="e02")
                            nc.scalar.activation(e02[:, 0:ncc * 4],
                                                 erp[:, 0:ncc * 4],
                                                 AF.Copy, scale=0.2)
                            lr = smp.tile([P, CALL_CH * 4], fp32, tag="lr")
                            nc.vector.tensor_tensor(
                                out=lr[:, 0:ncc * 4], in0=e02[:, 0:ncc * 4],
                                in1=erp[:, 0:ncc * 4], op=OP.max)
                            V = gp.tile([P, CALL_CH, 132], fp16, tag="V")
                            nc.scalar.activation(
                                V[:, 0:ncc, 128:132],
                                lr[:, 0:ncc * 4].rearrange(
                                    "p (c h) -> p c h", h=4),
                                AF.Exp)
                            nc.vector.tensor_tensor(
                                out=V[:, 0:ncc, 0:128].rearrange(
                                    "p c (h f) -> p c h f", f=F),
                                in0=Gt[:, 0:ncc, 0:128].rearrange(
                                    "p c (h f) -> p c h f", f=F),
                                in1=V[:, 0:ncc, 128:132]
                                    .unsqueeze(3).to_broadcast([P, ncc, 4, F]),
                                op=OP.mult)
                            for c in range(ncc):
                                gc = c0 + c
                                b = int(blk_of[r][gc])
                                sb_, lb = int(starts[b, r]), int(l[b, r])
                                if b not in pm_tiles:
                                    pm_tiles[b] = psp.tile(
                                        [P, 136], fp32, space="PSUM",
                                        tag="pm", name=f"pm{L}_{r}_{b}")
                                nc.tensor.matmul(pm_tiles[b][:, 0:132],
                                                 OE[:, c, :], V[:, c, :],
                                                 start=(gc == sb_),
                                                 stop=(gc == sb_ + lb - 1))
                                if gc == sb_ + lb - 1:
                                    if k == 0:
                                        nc.vector.tensor_copy(
                                            acc_g[:, b - g * GS, :],
                                            pm_tiles[b][:, 0:132])
                                    else:
                                        nc.vector.tensor_tensor(
                                            out=acc_g[:, b - g * GS, :],
                                            in0=acc_g[:, b - g * GS, :],
                                            in1=pm_tiles[b][:, 0:132],
                                            op=OP.add)
                                    del pm_tiles[b]
                    emit_epilogue(L, g, acc_g)
    nc.compile()
    return nc


def kernel(**inputs):
    feats_f32 = np.asarray(inputs["features"], np.float32)
    src = np.asarray(inputs["src"]).astype(np.int64)
    dst = np.asarray(inputs["dst"]).astype(np.int64)
    n_nodes = feats_f32.shape[0]

    core_nodes, sched, data = _preprocess(src, dst, n_nodes)

    # weights: Wcat[L] = [W | Wl | Wr] with Wl = sum_f W[:,h,f]*al[h,f]
    wcat = np.zeros((3, D, 136), np.float16)
    for L, (wn, an, bn) in enumerate([("W0", "al0", "ar0"),
                                      ("W1", "al1", "ar1"),
                                      ("W2", "al2", "ar2")]):
        W = np.asarray(inputs[wn], np.float32)
        al = np.asarray(inputs[an], np.float32)
        ar = np.asarray(inputs[bn], np.float32)
        Wh = W.reshape(D, HEADS, F)
        wcat[L, :, 0:128] = W.astype(np.float16)
        wcat[L, :, 128:132] = np.einsum("dhf,hf->dh", Wh, al).astype(np.float16)
        wcat[L, :, 132:136] = np.einsum("dhf,hf->dh", Wh, ar).astype(np.float16)

    key = (n_nodes, src.shape[0])
    if key not in _CACHE:
        _CACHE[key] = _build(sched)
    nc = _CACHE[key]

    in_maps = []
    for c in range(NCORES):
        f16 = np.zeros((G * P, D), np.float16)
        nodes_c = core_nodes[c]
        f16[:len(nodes_c)] = feats_f32[nodes_c].astype(np.float16)
        in_maps.append({
            "feats": f16,
            "wcat": wcat,
            "idx_t": data["idx_tiles"][c],
            "dcol_t": data["dcol"][c],
        })

    global LAST_PREP
    if PREPARE_ONLY:
        LAST_PREP = (nc, in_maps, core_nodes)
        return np.zeros((n_nodes, F), np.float32)

    import os
    trace = bool(int(os.environ.get("TRN_KERNEL_TRACE", "0")))
    res = run_bass_kernel_spmd(nc, in_maps, core_ids=list(range(NCORES)),
                               trace=trace)
    global LAST_RESULTS
    LAST_RESULTS = res
    out = np.zeros((n_nodes, F), np.float32)
    for c in range(NCORES):
        nodes_c = core_nodes[c]
        out[nodes_c] = res.results[c]["out_sh"][:len(nodes_c)]
    return out


# revision 24
# speedup vs baseline: 1.2065x; 1.0463x over previous
"""3-layer GAT on trn2, 8 NeuronCores (SPMD), group-pipelined.

Strategy:
- Nodes are snake-dealt by in-degree to 8 cores (12500 real rows each,
  padded to 12544 = 98 blocks of 128).  Table rows are GROUP-major:
  7 groups x (8 cores x 1792 rows), so each group's AllGather writes a
  contiguous 14336-row slice and collectives pipeline into the edge
  phase of the same / previous layer.
- Per layer, per group of 14 dst blocks: 4 range passes of dma_gather
  calls (<=1024 edges, int16 idx per 32768-row base slice; 512B rows
  [feat16|el|er]), then epilogue (softmax denom, residual, ELU or head
  mean), next-layer transform (PE transpose + matmul, no DMA
  transpose), bounce write and this group's AllGather.
- Per 128-edge chunk: OE one-hot from iota vs dst_local (DVE
  tensor_scalar), OT = PE-transpose(OE) + ACT copy; e = el + er via two
  PE matmuls into PSUM; ACT Lrelu(alpha=.2) + ACT Exp -> w; DVE scales
  feat into V; OE^T @ V accumulates into the block's PSUM tile across
  all 4 ranges; one copy into the group accumulator at block end.
"""
import numpy as np

import concourse.bacc as bacc
import concourse.bass as bass
import concourse.mybir as mybir
import concourse.tile as tile
from concourse.bass_utils import run_bass_kernel_spmd

P = 128
NCORES = 8
HEADS = 4
F = 32
D = 128            # feature dim (all layers)
TW = 256           # table row elems fp16 (512B)
CALL_CH = 8        # chunks per gather call (1024 idxs)
SEGW = 16          # max block segments per gather call
GS = 14            # blocks per group
NG = 7             # groups (G = 98 blocks)
G = NG * GS
SLICE = GS * P     # 1792 rows per (core, group)
TGW = NCORES * SLICE  # 14336 table rows per group = one gather base slice
NR = NG            # idx ranges == table groups
fp16 = mybir.dt.float16
fp32 = mybir.dt.float32
AF = mybir.ActivationFunctionType
OP = mybir.AluOpType

_CACHE = {}
LAST_RESULTS = None
PREPARE_ONLY = False   # bench hook: stash (nc, in_maps) and skip HW run
LAST_PREP = None
QUEUES = 4             # SWDGE queues for dma_gather round-robin
DMA_SCRATCH = 32768    # SWDGE descriptor ring bytes (2 calls deep)
OE_TS = False          # per-chunk tensor_scalar OE (SLOW on hw)


# ----------------------------------------------------------------- host side
def _preprocess(src, dst, n_nodes):
    """Node permutation, group-major table rows, per-core idx tiles."""
    percore = n_nodes // NCORES
    assert percore * NCORES == n_nodes and percore <= NG * SLICE
    shard = NG * SLICE                                         # 12544
    deg = np.bincount(dst, minlength=n_nodes)
    order = np.argsort(-deg, kind="stable")                    # desc degree
    core_of_pos = np.tile(np.concatenate([np.arange(NCORES),
                                          np.arange(NCORES)[::-1]]),
                          (n_nodes + 2 * NCORES - 1) // (2 * NCORES))[:n_nodes]
    node_core = np.empty(n_nodes, np.int64)
    node_core[order] = core_of_pos
    pos = np.empty(n_nodes, np.int64)
    core_nodes = []
    for c in range(NCORES):
        nodes_c = order[node_core[order] == c]
        assert len(nodes_c) == percore
        core_nodes.append(nodes_c)
        pos[nodes_c] = np.arange(percore)
    # group-major table row of each node
    trow = (pos // SLICE) * TGW + node_core * SLICE + (pos % SLICE)

    e_core = node_core[dst]
    p_dst = pos[dst]
    e_blk = p_dst // P
    e_dloc = (p_dst % P).astype(np.int16)
    e_rng = trow[src] // TGW
    e_idx = (trow[src] % TGW).astype(np.int16)

    counts = np.zeros((NCORES, G, NR), np.int64)
    np.add.at(counts, (e_core, e_blk, e_rng), 1)
    cap = counts.max(axis=0)                   # exact slots per (b, r)
    for b in range(G):
        if cap[b].sum() == 0:
            cap[b, 0] = 1
    # block slot offsets within each (g, r) stream; chunk counts
    off = np.zeros((G, NR), np.int64)
    S = np.zeros((NG, NR), np.int64)
    for g in range(NG):
        for r in range(NR):
            o = 0
            for b in range(g * GS, (g + 1) * GS):
                off[b, r] = o
                o += cap[b, r]
            S[g, r] = o
    NCH = -(-S // P)                           # chunks per (g, r)
    gstart = np.zeros((NG, NR), np.int64)      # chunk offset of group in range
    gstart[1:] = np.cumsum(NCH[:-1], axis=0)
    chunks_r = NCH.sum(axis=0)

    # segments + calls per (g, r) (shared across cores)
    group_calls = [[] for _ in range(NR)]  # [r][g]->[(ncc,idxcol,[segs/chunk])]
    cols_r = [0] * NR
    segn_r = [0] * NR
    for r in range(NR):
        col = 0
        scol = 0
        for g in range(NG):
            nch = int(NCH[g, r])
            chunk_segs = []
            for j in range(nch):
                lo, hi = j * P, (j + 1) * P
                segs = []
                for b in range(g * GS, (g + 1) * GS):
                    cb = int(cap[b, r])
                    if cb == 0:
                        continue
                    s0, s1 = int(off[b, r]), int(off[b, r]) + cb
                    if s0 < hi and s1 > lo:
                        segs.append((b, scol,
                                     s0 >= lo,            # br_start
                                     s1 <= hi))           # br_stop
                        scol += 1
                if not segs:
                    segs.append((g * GS, scol, False, False))
                    scol += 1
                chunk_segs.append(segs)
            calls = []
            for k in range((nch + CALL_CH - 1) // CALL_CH):
                ncc = min(CALL_CH, nch - k * CALL_CH)
                cs = chunk_segs[k * CALL_CH:k * CALL_CH + ncc]
                assert sum(len(x) for x in cs) <= SEGW
                calls.append((ncc, col, cs))
                col += ncc * P // 16
            group_calls[r].append(calls)
        cols_r[r] = col
        segn_r[r] = scol
    colsmax = max(cols_r)
    segmax = max(segn_r)

    idx_tiles = np.zeros((NCORES, NR, P, colsmax), np.int16)
    dcol = np.full((NCORES, NR, P, segmax), 240, np.float16)

    eo = np.lexsort((e_rng, e_blk, e_core))
    es, eb, er_, ei, ed = (x[eo] for x in (e_core, e_blk, e_rng, e_idx, e_dloc))
    grp = es * (G * NR) + eb * NR + er_
    sort_count = np.bincount(grp, minlength=NCORES * G * NR)
    within = np.arange(len(eo)) - np.repeat(
        np.concatenate([[0], np.cumsum(sort_count)[:-1]]), sort_count)
    slot = gstart[eb // GS, er_] * P + off[eb, er_] + within

    for c in range(NCORES):
        m = es == c
        s, r, iv, dv = slot[m], er_[m], ei[m], ed[m]
        for rr in range(NR):
            mm = r == rr
            nch = int(chunks_r[rr])
            flat_i = np.zeros(nch * P, np.int16)          # pad -> row 0 (real)
            flat_d = np.full(nch * P, 240, np.int16)      # pad dst -> no one-hot
            flat_i[s[mm]] = iv[mm]
            flat_d[s[mm]] = dv[mm]
            for g in range(NG):
                base = int(gstart[g, rr]) * P
                for (ncc, col, cs) in group_calls[rr][g]:
                    pass
                for j, segs in enumerate(
                        [x for (ncc, col, cs) in group_calls[rr][g]
                         for x in cs]):
                    lo = base + j * P
                    dseg = flat_d[lo:lo + P].copy()
                    for (b, scol, _, _) in segs:
                        dv2 = dseg.copy()
                        s0 = int(off[b, rr]) - j * P
                        s1 = s0 + int(cap[b, rr])
                        posn = np.arange(P)
                        dv2[(posn < s0) | (posn >= s1)] = 240
                        dcol[c, rr, :, scol] = dv2.astype(np.float16)
                for (ncc, col, cs) in group_calls[rr][g]:
                    pass
            for g in range(NG):
                base = int(gstart[g, rr]) * P
                j0 = 0
                for (ncc, col, cs) in group_calls[rr][g]:
                    ni = ncc * P
                    seg = flat_i[base + j0 * P:base + j0 * P + ni]
                    idx_tiles[c, rr, :, col:col + ni // 16] = np.tile(
                        seg.reshape(ni // 16, 16).T, (8, 1))
                    j0 += ncc

    sched = dict(shard=shard, percore=percore, cap=cap,
                 chunks_r=chunks_r, segmax=segmax, colsmax=colsmax,
                 group_calls=group_calls)
    data = dict(idx_tiles=idx_tiles, dcol=dcol)
    return core_nodes, sched, data


# ------------------------------------------------------------- device program
def _build(sched):
    segmax, colsmax = sched["segmax"], sched["colsmax"]
    group_calls = sched["group_calls"]
    percore = sched["percore"]
    GR = G * P                                                 # 12544
    NT = NCORES * GR                                           # 100352
    # dummy h rows (same on every core): local positions [percore, GR)
    dum_blk = (percore % SLICE) // P                           # block in g6
    dum_row = percore % P

    nc = bacc.Bacc("TRN2", target_bir_lowering=False, debug=False,
                   num_devices=NCORES, num_swdge_queues=QUEUES,
                   dynamic_dma_scratch_size=DMA_SCRATCH)
    feats = nc.dram_tensor("feats", [GR, D], fp16, kind="ExternalInput")
    wcat = nc.dram_tensor("wcat", [3, D, 136], fp16, kind="ExternalInput")
    idx_t = nc.dram_tensor("idx_t", [NR, P, colsmax], mybir.dt.int16,
                           kind="ExternalInput")
    dcol_t = nc.dram_tensor("dcol_t", [NR, P, segmax], fp16,
                            kind="ExternalInput")
    out_sh = nc.dram_tensor("out_sh", [GR, F], fp32, kind="ExternalOutput")

    qctr = [0]

    with tile.TileContext(nc) as tc:
        with (
            tc.tile_pool(name="const", bufs=1) as cp,
            tc.tile_pool(name="dram", bufs=1, space="DRAM") as dramp,
            tc.tile_pool(name="stf", bufs=2) as stfp,
            tc.tile_pool(name="hsb", bufs=2) as hsbp,
            tc.tile_pool(name="ht", bufs=3) as htp,
            tc.tile_pool(name="gath", bufs=10) as gp,
            tc.tile_pool(name="oh", bufs=6) as ohp,
            tc.tile_pool(name="ot", bufs=20) as otp,
            tc.tile_pool(name="sm", bufs=6) as smp,
            tc.tile_pool(name="acc", bufs=2) as accp,
            tc.tile_pool(name="epi", bufs=1) as epip,
            tc.tile_pool(name="ps", bufs=4, space="PSUM") as psp,
            tc.tile_pool(name="pst", bufs=2, space="PSUM") as pstp,
            tc.tile_pool(name="pse", bufs=2, space="PSUM") as psep,
        ):
            h1 = dramp.tile([GR, D], fp16)
            h2 = dramp.tile([GR, D], fp16)
            bounce = dramp.tile([GR, TW], fp16)
            tables = [[dramp.tile([TGW, TW], fp16, addr_space="Shared",
                                  name=f"table{L}_{g}") for g in range(NG)]
                      for L in range(3)]

            # constants
            iota = cp.tile([P, P], fp16)
            nc.gpsimd.iota(iota[:], pattern=[[1, P]], base=0,
                           channel_multiplier=0,
                           allow_small_or_imprecise_dtypes=True)
            iota8 = cp.tile([P, SEGW, P], fp16)
            nc.gpsimd.iota(iota8[:], pattern=[[0, SEGW], [1, P]], base=0,
                           channel_multiplier=0,
                           allow_small_or_imprecise_dtypes=True)
            chiota = cp.tile([P, 1], fp32)
            nc.gpsimd.iota(chiota[:], pattern=[[0, 1]], base=0,
                           channel_multiplier=1,
                           allow_small_or_imprecise_dtypes=True)
            ident = cp.tile([P, P], fp16)
            nc.vector.tensor_scalar(ident[:], iota[:], chiota[:], None,
                                    op0=OP.is_equal)
            wcs = []
            for L in range(3):
                wc = cp.tile([D, 136], fp16, tag=f"wc{L}", name=f"wc{L}")
                nc.sync.dma_start(wc[:], wcat[L])
                wcs.append(wc)

            # resident idx / dcol tiles (constant across layers)
            idxs_res, dcol_res = [], []
            for r in range(NR):
                it = cp.tile([P, colsmax], mybir.dt.int16, tag=f"idxr{r}",
                             name=f"idxr{r}")
                nc.sync.dma_start(it[:], idx_t[r])
                idxs_res.append(it)
                dc = cp.tile([P, segmax], fp16, tag=f"dcolr{r}",
                             name=f"dcolr{r}")
                nc.sync.dma_start(dc[:], dcol_t[r])
                dcol_res.append(dc)

            # per-layer er tables [P, G, 4] (small, persistent)
            er_all = [cp.tile([P, G, 4], fp16, tag=f"erall{L}",
                              name=f"er{L}") for L in range(3)]

            h_of = {0: feats[:], 1: h1[:], 2: h2[:]}

            def emit_transform(L, g, hsb):
                """Group g: stf = h @ [W|Wl|Wr]; bounce + AllGather slice."""
                stf_g = stfp.tile([P, GS, 136], fp16, tag="stf")
                for bi in range(GS):
                    pt = pstp.tile([P, P], fp16, space="PSUM", tag="tp")
                    nc.tensor.transpose(pt[:], hsb[:, bi, :], ident[:])
                    hT = htp.tile([P, P], fp16, tag="hT")
                    nc.scalar.activation(hT[:], pt[:], AF.Copy)
                    pf = psp.tile([P, 136], fp32, space="PSUM", tag="pm")
                    nc.tensor.matmul(pf[:], hT[:], wcs[L][:],
                                     start=True, stop=True)
                    nc.vector.tensor_copy(stf_g[:, bi, :], pf[:])
                nc.vector.tensor_copy(er_all[L][:, g * GS:(g + 1) * GS, :],
                                      stf_g[:, :, 132:136])
                nc.sync.dma_start(
                    bounce[g * SLICE:(g + 1) * SLICE, 0:136].rearrange(
                        "(b p) f -> p b f", p=P),
                    stf_g[:])
                nc.gpsimd.collective_compute(
                    "AllGather", OP.bypass,
                    replica_groups=[list(range(NCORES))],
                    ins=[bounce[g * SLICE:(g + 1) * SLICE, :].opt()],
                    outs=[tables[L][g][:].opt()])

            def emit_epilogue(L, g, acc_g):
                rec = epip.tile([P, GS, 4], fp32, tag="rec")
                nc.vector.tensor_scalar_add(rec[:], acc_g[:, :, 128:132],
                                            1e-6)
                nc.vector.reciprocal(rec[:], rec[:])
                av = epip.tile([P, GS, D], fp32, tag="av")
                nc.vector.tensor_tensor(
                    out=av[:].rearrange("p b (h f) -> p b h f", f=F),
                    in0=acc_g[:, :, 0:128].rearrange(
                        "p b (h f) -> p b h f", f=F),
                    in1=rec[:].unsqueeze(3).to_broadcast([P, GS, HEADS, F]),
                    op=OP.mult)
                if L >= 1:  # residual
                    hres = epip.tile([P, GS, D], fp16, tag="hres")
                    nc.sync.dma_start(
                        hres[:],
                        h_of[L][g * SLICE:(g + 1) * SLICE, :].rearrange(
                            "(b p) d -> p b d", p=P))
                    nc.vector.tensor_tensor(out=av[:], in0=av[:],
                                            in1=hres[:], op=OP.add)
                if L < 2:
                    # ELU: relu(x) + exp(min(x,0)) - 1
                    relu = epip.tile([P, GS * D], fp16, tag="relu")
                    flat = av[:].rearrange("p b d -> p (b d)")
                    nc.vector.tensor_scalar_max(relu[:], flat, 0.0)
                    nc.vector.tensor_scalar_min(flat, flat, 0.0)
                    ex = epip.tile([P, GS * D], fp16, tag="ex")
                    nc.scalar.activation(ex[:], flat, AF.Exp)
                    hnext = epip.tile([P, GS, D], fp16, tag="hnext")
                    nc.vector.scalar_tensor_tensor(
                        hnext[:].rearrange("p b d -> p (b d)"),
                        ex[:], -1.0, relu[:], op0=OP.add, op1=OP.add)
                    nc.sync.dma_start(
                        h_of[L + 1][g * SLICE:(g + 1) * SLICE, :].rearrange(
                            "(b p) d -> p b d", p=P),
                        hnext[:])
                    emit_transform(L + 1, g, hnext)
                else:
                    o32 = epip.tile([P, GS, F], fp32, tag="o32")
                    avh = av[:].rearrange("p b (h f) -> p b h f", f=F)
                    nc.vector.tensor_tensor(out=o32[:], in0=avh[:, :, 0, :],
                                            in1=avh[:, :, 1, :], op=OP.add)
                    nc.vector.tensor_tensor(out=o32[:], in0=o32[:],
                                            in1=avh[:, :, 2, :], op=OP.add)
                    nc.vector.tensor_tensor(out=o32[:], in0=o32[:],
                                            in1=avh[:, :, 3, :], op=OP.add)
                    nc.vector.tensor_scalar_mul(o32[:], o32[:], 0.25)
                    nc.sync.dma_start(
                        out_sh[g * SLICE:(g + 1) * SLICE, :].rearrange(
                            "(b p) f -> p b f", p=P),
                        o32[:])

            # ---- layer-0 prologue: per-group transform + collective
            for g in range(NG):
                hsb = hsbp.tile([P, GS, D], fp16, tag="hsb")
                nc.sync.dma_start(
                    hsb[:],
                    feats[g * SLICE:(g + 1) * SLICE, :].rearrange(
                        "(b p) d -> p b d", p=P))
                emit_transform(0, g, hsb)

            # ---- layers
            for L in range(3):
                for g in range(NG):
                    acc_g = accp.tile([P, GS, 132], fp32, tag="acc")
                    blk_seen = set()
                    for k in range(NR):
                        r = (g + k) % NR
                        pm_tiles = {}
                        for (ncc, col, cs) in group_calls[r][g]:
                            ni = ncc * P
                            nseg = sum(len(x) for x in cs)
                            scol0 = cs[0][0][1]
                            Gt = gp.tile([P, CALL_CH, TW], fp16, tag="G")
                            nc.gpsimd.dma_gather(
                                Gt[:, 0:ncc, :],
                                tables[L][r][:],
                                idxs_res[r][:, col:col + ni // 16],
                                num_idxs=ni, num_idxs_reg=ni,
                                elem_size=TW,
                                queue_num=qctr[0] % QUEUES,
                                single_packet=True)
                            qctr[0] += 1
                            OE = ohp.tile([P, SEGW, P], fp16, tag="OE")
                            nc.vector.tensor_tensor(
                                out=OE[:, 0:nseg, :],
                                in0=iota8[:, 0:nseg, :],
                                in1=dcol_res[r][:, scol0:scol0 + nseg]
                                    .unsqueeze(2).to_broadcast([P, nseg, P]),
                                op=OP.is_equal)
                            erp = psep.tile([P, CALL_CH * 4], fp32,
                                            space="PSUM", tag="er")
                            nc.tensor.matmul(
                                erp[:, 0:ncc * 4].rearrange(
                                    "p (c h) -> p c h", h=4),
                                ident[:], Gt[:, 0:ncc, 128:132],
                                start=True, stop=False)
                            si = 0
                            for c in range(ncc):
                                for t, (b, scol, bst, bsp) in enumerate(cs[c]):
                                    pt = pstp.tile([P, P], fp16, space="PSUM",
                                                   tag="tp")
                                    nc.tensor.transpose(pt[:], OE[:, si, :],
                                                        ident[:])
                                    OTs = otp.tile([P, P], fp16, tag="OT")
                                    nc.scalar.activation(OTs[:], pt[:],
                                                         AF.Copy)
                                    nc.tensor.matmul(
                                        erp[:, c * 4:(c + 1) * 4],
                                        OTs[:], er_all[L][:, b, :],
                                        start=False,
                                        stop=(t == len(cs[c]) - 1))
                                    si += 1
                            e02 = smp.tile([P, CALL_CH * 4], fp32,
                                           tag="e02")
                            nc.scalar.activation(e02[:, 0:ncc * 4],
                                                 erp[:, 0:ncc * 4],
                                                 AF.Copy, scale=0.2)
                            lr = smp.tile([P, CALL_CH * 4], fp32, tag="lr")
                            nc.vector.tensor_tensor(
                                out=lr[:, 0:ncc * 4], in0=e02[:, 0:ncc * 4],
                                in1=erp[:, 0:ncc * 4], op=OP.max)
                            V = gp.tile([P, CALL_CH, 132], fp16, tag="V")
                            nc.scalar.activation(
                                V[:, 0:ncc, 128:132],
                                lr[:, 0:ncc * 4].rearrange(
                                    "p (c h) -> p c h", h=4),
                                AF.Exp)
                            nc.vector.tensor_tensor(
                                out=V[:, 0:ncc, 0:128].rearrange(
                                    "p c (h f) -> p c h f", f=F),
                                in0=Gt[:, 0:ncc, 0:128].rearrange(
                                    "p c (h f) -> p c h f", f=F),
                                in1=V[:, 0:ncc, 128:132]
                                    .unsqueeze(3).to_broadcast([P, ncc, 4, F]),
                                op=OP.mult)
                            si = 0
                            for c in range(ncc):
                                for (b, scol, bst, bsp) in cs[c]:
                                    if b not in pm_tiles:
                                        pm_tiles[b] = psp.tile(
                                            [P, 136], fp32, space="PSUM",
                                            tag="pm", name=f"pm{L}_{r}_{b}")
                                    nc.tensor.matmul(pm_tiles[b][:, 0:132],
                                                     OE[:, si, :], V[:, c, :],
                                                     start=bst, stop=bsp)
                                    si += 1
                                    if bsp:
                                        if b in blk_seen:
                                            nc.vector.tensor_tensor(
                                                out=acc_g[:, b - g * GS, :],
                                                in0=acc_g[:, b - g * GS, :],
                                                in1=pm_tiles[b][:, 0:132],
                                                op=OP.add)
                                        else:
                                            nc.vector.tensor_copy(
                                                acc_g[:, b - g * GS, :],
                                                pm_tiles[b][:, 0:132])
                                            blk_seen.add(b)
                                        del pm_tiles[b]
                    emit_epilogue(L, g, acc_g)
    nc.compile()
    return nc


def kernel(**inputs):
    feats_f32 = np.asarray(inputs["features"], np.float32)
    src = np.asarray(inputs["src"]).astype(np.int64)
    dst = np.asarray(inputs["dst"]).astype(np.int64)
    n_nodes = feats_f32.shape[0]

    core_nodes, sched, data = _preprocess(src, dst, n_nodes)

    # weights: Wcat[L] = [W | Wl | Wr] with Wl = sum_f W[:,h,f]*al[h,f]
    wcat = np.zeros((3, D, 136), np.float16)
    for L, (wn, an, bn) in enumerate([("W0", "al0", "ar0"),
                                      ("W1", "al1", "ar1"),
                                      ("W2", "al2", "ar2")]):
        W = np.asarray(inputs[wn], np.float32)
        al = np.asarray(inputs[an], np.float32)
        ar = np.asarray(inputs[bn], np.float32)
        Wh = W.reshape(D, HEADS, F)
        wcat[L, :, 0:128] = W.astype(np.float16)
        wcat[L, :, 128:132] = np.einsum("dhf,hf->dh", Wh, al).astype(np.float16)
        wcat[L, :, 132:136] = np.einsum("dhf,hf->dh", Wh, ar).astype(np.float16)

    key = (n_nodes, src.shape[0])
    if key not in _CACHE:
        _CACHE[key] = _build(sched)
    nc = _CACHE[key]

    in_maps = []
    for c in range(NCORES):
        f16 = np.zeros((G * P, D), np.float16)
        nodes_c = core_nodes[c]
        f16[:len(nodes_c)] = feats_f32[nodes_c].astype(np.float16)
        in_maps.append({
            "feats": f16,
            "wcat": wcat,
            "idx_t": data["idx_tiles"][c],
            "dcol_t": data["dcol"][c],
        })

    global LAST_PREP
    if PREPARE_ONLY:
        LAST_PREP = (nc, in_maps, core_nodes)
        return np.zeros((n_nodes, F), np.float32)

    import os
    trace = bool(int(os.environ.get("TRN_KERNEL_TRACE", "0")))
    res = run_bass_kernel_spmd(nc, in_maps, core_ids=list(range(NCORES)),
                               trace=trace)
    global LAST_RESULTS
    LAST_RESULTS = res
    out = np.zeros((n_nodes, F), np.float32)
    for c in range(NCORES):
        nodes_c = core_nodes[c]
        out[nodes_c] = res.results[c]["out_sh"][:len(nodes_c)]
    return out


# revision 26
# speedup vs baseline: 1.6283x; 1.3496x over previous
"""3-layer GAT on trn2, 8 NeuronCores (SPMD), group-pipelined.

Strategy:
- Nodes are snake-dealt by in-degree to 8 cores (12500 real rows each,
  padded to 12544 = 98 blocks of 128).  Table rows are GROUP-major:
  7 groups x (8 cores x 1792 rows); each group is its own Shared DRAM
  tensor written by exactly one AllGather, so collectives pipeline into
  the edge phase of the same / previous layer.
- Per layer, per group of 14 dst blocks: 7 gather passes (one per table
  group, rotated by dst-group to dodge collective latency), then
  epilogue (softmax denom + eps, residual, ELU or head mean),
  next-layer transform (PE transpose + matmul; no DMA transpose),
  h/bounce stores and this group's AllGather.
- Chunks (128 gathered edges, 512B rows [feat16|el|er]) span block
  boundaries: per (block, range) capacity = max edge count over cores
  (no ceil-to-128), cutting gather descriptors ~15%.  Boundary chunks
  emit one one-hot segment per block.
- Per chunk: OE one-hot from iota vs dst_local (bulk DVE is_equal per
  call), OT = PE-transpose(OE) + ACT copy; e = el + er via one batched
  + per-segment PE matmuls into PSUM; ACT copy(0.2x) + DVE max +
  ACT Exp -> w (Copy/Exp share one ACT table set); DVE scales feat
  into V; OE_seg^T @ V accumulates per-block PSUM across a pass, added
  into the per-group SBUF accumulator.
"""
import numpy as np

import concourse.bacc as bacc
import concourse.bass as bass
import concourse.mybir as mybir
import concourse.tile as tile
from concourse.bass_utils import run_bass_kernel_spmd

P = 128
NCORES = 8
HEADS = 4
F = 32
D = 128            # feature dim (all layers)
TW = 256           # table row elems fp16 (512B)
CALL_CH = 8        # chunks per gather call (1024 idxs)
SEGW = 16          # max block segments per gather call
GS = 14            # blocks per group
NG = 7             # groups (G = 98 blocks)
G = NG * GS
SLICE = GS * P     # 1792 rows per (core, group)
TGW = NCORES * SLICE  # 14336 table rows per group = one gather base slice
NR = NG            # idx ranges == table groups
fp16 = mybir.dt.float16
fp32 = mybir.dt.float32
AF = mybir.ActivationFunctionType
OP = mybir.AluOpType

_CACHE = {}
LAST_RESULTS = None
PREPARE_ONLY = False   # bench hook: stash (nc, in_maps) and skip HW run
LAST_PREP = None
QUEUES = 4             # SWDGE queues for dma_gather round-robin
DMA_SCRATCH = 32768    # SWDGE descriptor ring bytes (2 calls deep)
OE_TS = False          # per-chunk tensor_scalar OE (SLOW on hw)


# ----------------------------------------------------------------- host side
def _preprocess(src, dst, n_nodes):
    """Node permutation, group-major table rows, per-core idx tiles."""
    percore = n_nodes // NCORES
    assert percore * NCORES == n_nodes and percore <= NG * SLICE
    shard = NG * SLICE                                         # 12544
    deg = np.bincount(dst, minlength=n_nodes)
    order = np.argsort(-deg, kind="stable")                    # desc degree
    core_of_pos = np.tile(np.concatenate([np.arange(NCORES),
                                          np.arange(NCORES)[::-1]]),
                          (n_nodes + 2 * NCORES - 1) // (2 * NCORES))[:n_nodes]
    node_core = np.empty(n_nodes, np.int64)
    node_core[order] = core_of_pos
    pos = np.empty(n_nodes, np.int64)
    core_nodes = []
    for c in range(NCORES):
        nodes_c = order[node_core[order] == c]
        assert len(nodes_c) == percore
        core_nodes.append(nodes_c)
        pos[nodes_c] = np.arange(percore)
    # group-major table row of each node
    trow = (pos // SLICE) * TGW + node_core * SLICE + (pos % SLICE)

    e_core = node_core[dst]
    p_dst = pos[dst]
    e_blk = p_dst // P
    e_dloc = (p_dst % P).astype(np.int16)
    e_rng = trow[src] // TGW
    e_idx = (trow[src] % TGW).astype(np.int16)

    counts = np.zeros((NCORES, G, NR), np.int64)
    np.add.at(counts, (e_core, e_blk, e_rng), 1)
    cap = counts.max(axis=0)                   # exact slots per (b, r)
    for b in range(G):
        if cap[b].sum() == 0:
            cap[b, 0] = 1
    # block slot offsets within each (g, r) stream; chunk counts
    off = np.zeros((G, NR), np.int64)
    S = np.zeros((NG, NR), np.int64)
    for g in range(NG):
        for r in range(NR):
            o = 0
            for b in range(g * GS, (g + 1) * GS):
                off[b, r] = o
                o += cap[b, r]
            S[g, r] = o
    NCH = -(-S // P)                           # chunks per (g, r)
    gstart = np.zeros((NG, NR), np.int64)      # chunk offset of group in range
    gstart[1:] = np.cumsum(NCH[:-1], axis=0)
    chunks_r = NCH.sum(axis=0)

    # segments + calls per (g, r) (shared across cores)
    group_calls = [[] for _ in range(NR)]  # [r][g]->[(ncc,idxcol,[segs/chunk])]
    cols_r = [0] * NR
    segn_r = [0] * NR
    for r in range(NR):
        col = 0
        scol = 0
        for g in range(NG):
            nch = int(NCH[g, r])
            chunk_segs = []
            for j in range(nch):
                lo, hi = j * P, (j + 1) * P
                segs = []
                for b in range(g * GS, (g + 1) * GS):
                    cb = int(cap[b, r])
                    if cb == 0:
                        continue
                    s0, s1 = int(off[b, r]), int(off[b, r]) + cb
                    if s0 < hi and s1 > lo:
                        segs.append((b, scol,
                                     s0 >= lo,            # br_start
                                     s1 <= hi))           # br_stop
                        scol += 1
                if not segs:
                    segs.append((g * GS, scol, False, False))
                    scol += 1
                chunk_segs.append(segs)
            calls = []
            for k in range((nch + CALL_CH - 1) // CALL_CH):
                ncc = min(CALL_CH, nch - k * CALL_CH)
                cs = chunk_segs[k * CALL_CH:k * CALL_CH + ncc]
                assert sum(len(x) for x in cs) <= SEGW
                calls.append((ncc, col, cs))
                col += ncc * P // 16
            group_calls[r].append(calls)
        cols_r[r] = col
        segn_r[r] = scol
    colsmax = max(cols_r)
    segmax = max(segn_r)

    idx_tiles = np.zeros((NCORES, NR, P, colsmax), np.int16)
    dcol = np.full((NCORES, NR, P, segmax), 240, np.float16)

    eo = np.lexsort((e_rng, e_blk, e_core))
    es, eb, er_, ei, ed = (x[eo] for x in (e_core, e_blk, e_rng, e_idx, e_dloc))
    grp = es * (G * NR) + eb * NR + er_
    sort_count = np.bincount(grp, minlength=NCORES * G * NR)
    within = np.arange(len(eo)) - np.repeat(
        np.concatenate([[0], np.cumsum(sort_count)[:-1]]), sort_count)
    slot = gstart[eb // GS, er_] * P + off[eb, er_] + within

    for c in range(NCORES):
        m = es == c
        s, r, iv, dv = slot[m], er_[m], ei[m], ed[m]
        for rr in range(NR):
            mm = r == rr
            nch = int(chunks_r[rr])
            flat_i = np.zeros(nch * P, np.int16)          # pad -> row 0 (real)
            flat_d = np.full(nch * P, 240, np.int16)      # pad dst -> no one-hot
            flat_i[s[mm]] = iv[mm]
            flat_d[s[mm]] = dv[mm]
            for g in range(NG):
                base = int(gstart[g, rr]) * P
                for (ncc, col, cs) in group_calls[rr][g]:
                    pass
                for j, segs in enumerate(
                        [x for (ncc, col, cs) in group_calls[rr][g]
                         for x in cs]):
                    lo = base + j * P
                    dseg = flat_d[lo:lo + P].copy()
                    for (b, scol, _, _) in segs:
                        dv2 = dseg.copy()
                        s0 = int(off[b, rr]) - j * P
                        s1 = s0 + int(cap[b, rr])
                        posn = np.arange(P)
                        dv2[(posn < s0) | (posn >= s1)] = 240
                        dcol[c, rr, :, scol] = dv2.astype(np.float16)
                for (ncc, col, cs) in group_calls[rr][g]:
                    pass
            for g in range(NG):
                base = int(gstart[g, rr]) * P
                j0 = 0
                for (ncc, col, cs) in group_calls[rr][g]:
                    ni = ncc * P
                    seg = flat_i[base + j0 * P:base + j0 * P + ni]
                    idx_tiles[c, rr, :, col:col + ni // 16] = np.tile(
                        seg.reshape(ni // 16, 16).T, (8, 1))
                    j0 += ncc

    sched = dict(shard=shard, percore=percore, cap=cap,
                 chunks_r=chunks_r, segmax=segmax, colsmax=colsmax,
                 group_calls=group_calls)
    data = dict(idx_tiles=idx_tiles, dcol=dcol)
    return core_nodes, sched, data


# ------------------------------------------------------------- device program
def _build(sched):
    segmax, colsmax = sched["segmax"], sched["colsmax"]
    group_calls = sched["group_calls"]
    percore = sched["percore"]
    GR = G * P                                                 # 12544
    NT = NCORES * GR                                           # 100352
    # dummy h rows (same on every core): local positions [percore, GR)
    dum_blk = (percore % SLICE) // P                           # block in g6
    dum_row = percore % P

    nc = bacc.Bacc("TRN2", target_bir_lowering=False, debug=False,
                   num_devices=NCORES, num_swdge_queues=QUEUES,
                   dynamic_dma_scratch_size=DMA_SCRATCH)
    feats = nc.dram_tensor("feats", [GR, D], fp16, kind="ExternalInput")
    wcat = nc.dram_tensor("wcat", [3, D, 136], fp16, kind="ExternalInput")
    idx_t = nc.dram_tensor("idx_t", [NR, P, colsmax], mybir.dt.int16,
                           kind="ExternalInput")
    dcol_t = nc.dram_tensor("dcol_t", [NR, P, segmax], fp16,
                            kind="ExternalInput")
    out_sh = nc.dram_tensor("out_sh", [GR, F], fp32, kind="ExternalOutput")

    qctr = [0]

    with tile.TileContext(nc) as tc:
        with (
            tc.tile_pool(name="const", bufs=1) as cp,
            tc.tile_pool(name="dram", bufs=1, space="DRAM") as dramp,
            tc.tile_pool(name="stf", bufs=2) as stfp,
            tc.tile_pool(name="hsb", bufs=2) as hsbp,
            tc.tile_pool(name="ht", bufs=3) as htp,
            tc.tile_pool(name="gath", bufs=10) as gp,
            tc.tile_pool(name="oh", bufs=6) as ohp,
            tc.tile_pool(name="ot", bufs=2) as otp,
            tc.tile_pool(name="sm", bufs=6) as smp,
            tc.tile_pool(name="acc", bufs=2) as accp,
            tc.tile_pool(name="epi", bufs=1) as epip,
            tc.tile_pool(name="ps", bufs=4, space="PSUM") as psp,
            tc.tile_pool(name="pst", bufs=2, space="PSUM") as pstp,
            tc.tile_pool(name="pse", bufs=2, space="PSUM") as psep,
        ):
            h1 = dramp.tile([GR, D], fp16)
            h2 = dramp.tile([GR, D], fp16)
            bounce = dramp.tile([GR, TW], fp16)
            tables = [[dramp.tile([TGW, TW], fp16, addr_space="Shared",
                                  name=f"table{L}_{g}") for g in range(NG)]
                      for L in range(3)]

            # constants
            iota = cp.tile([P, P], fp16)
            nc.gpsimd.iota(iota[:], pattern=[[1, P]], base=0,
                           channel_multiplier=0,
                           allow_small_or_imprecise_dtypes=True)
            iota8 = cp.tile([P, SEGW, P], fp16)
            nc.gpsimd.iota(iota8[:], pattern=[[0, SEGW], [1, P]], base=0,
                           channel_multiplier=0,
                           allow_small_or_imprecise_dtypes=True)
            chiota = cp.tile([P, 1], fp32)
            nc.gpsimd.iota(chiota[:], pattern=[[0, 1]], base=0,
                           channel_multiplier=1,
                           allow_small_or_imprecise_dtypes=True)
            ident = cp.tile([P, P], fp16)
            nc.vector.tensor_scalar(ident[:], iota[:], chiota[:], None,
                                    op0=OP.is_equal)
            wcs = []
            for L in range(3):
                wc = cp.tile([D, 136], fp16, tag=f"wc{L}", name=f"wc{L}")
                nc.sync.dma_start(wc[:], wcat[L])
                wcs.append(wc)

            # resident idx / dcol tiles (constant across layers)
            idxs_res, dcol_res = [], []
            for r in range(NR):
                it = cp.tile([P, colsmax], mybir.dt.int16, tag=f"idxr{r}",
                             name=f"idxr{r}")
                nc.sync.dma_start(it[:], idx_t[r])
                idxs_res.append(it)
                dc = cp.tile([P, segmax], fp16, tag=f"dcolr{r}",
                             name=f"dcolr{r}")
                nc.sync.dma_start(dc[:], dcol_t[r])
                dcol_res.append(dc)

            # per-layer er tables [P, G, 4] (small, persistent)
            er_all = [cp.tile([P, G, 4], fp16, tag=f"erall{L}",
                              name=f"er{L}") for L in range(3)]

            h_of = {0: feats[:], 1: h1[:], 2: h2[:]}

            def emit_transform(L, g, hsb):
                """Group g: stf = h @ [W|Wl|Wr]; bounce + AllGather slice."""
                stf_g = stfp.tile([P, GS, 136], fp16, tag="stf")
                for bi in range(GS):
                    pt = pstp.tile([P, P], fp16, space="PSUM", tag="tp")
                    nc.tensor.transpose(pt[:], hsb[:, bi, :], ident[:])
                    hT = htp.tile([P, P], fp16, tag="hT")
                    nc.scalar.activation(hT[:], pt[:], AF.Copy)
                    pf = psp.tile([P, 136], fp32, space="PSUM", tag="pm")
                    nc.tensor.matmul(pf[:], hT[:], wcs[L][:],
                                     start=True, stop=True)
                    nc.vector.tensor_copy(stf_g[:, bi, :], pf[:])
                nc.vector.tensor_copy(er_all[L][:, g * GS:(g + 1) * GS, :],
                                      stf_g[:, :, 132:136])
                nc.sync.dma_start(
                    bounce[g * SLICE:(g + 1) * SLICE, 0:136].rearrange(
                        "(b p) f -> p b f", p=P),
                    stf_g[:])
                nc.gpsimd.collective_compute(
                    "AllGather", OP.bypass,
                    replica_groups=[list(range(NCORES))],
                    ins=[bounce[g * SLICE:(g + 1) * SLICE, :].opt()],
                    outs=[tables[L][g][:].opt()])

            def emit_epilogue(L, g, acc_g):
                rec = epip.tile([P, GS, 4], fp32, tag="rec")
                nc.vector.tensor_scalar_add(rec[:], acc_g[:, :, 128:132],
                                            1e-6)
                nc.vector.reciprocal(rec[:], rec[:])
                av = epip.tile([P, GS, D], fp32, tag="av")
                nc.vector.tensor_tensor(
                    out=av[:].rearrange("p b (h f) -> p b h f", f=F),
                    in0=acc_g[:, :, 0:128].rearrange(
                        "p b (h f) -> p b h f", f=F),
                    in1=rec[:].unsqueeze(3).to_broadcast([P, GS, HEADS, F]),
                    op=OP.mult)
                if L >= 1:  # residual
                    hres = epip.tile([P, GS, D], fp16, tag="hres")
                    nc.sync.dma_start(
                        hres[:],
                        h_of[L][g * SLICE:(g + 1) * SLICE, :].rearrange(
                            "(b p) d -> p b d", p=P))
                    nc.vector.tensor_tensor(out=av[:], in0=av[:],
                                            in1=hres[:], op=OP.add)
                if L < 2:
                    # ELU: relu(x) + exp(min(x,0)) - 1
                    relu = epip.tile([P, GS * D], fp16, tag="relu")
                    flat = av[:].rearrange("p b d -> p (b d)")
                    nc.vector.tensor_scalar_max(relu[:], flat, 0.0)
                    nc.vector.tensor_scalar_min(flat, flat, 0.0)
                    ex = epip.tile([P, GS * D], fp16, tag="ex")
                    nc.scalar.activation(ex[:], flat, AF.Exp)
                    hnext = epip.tile([P, GS, D], fp16, tag="hnext")
                    nc.vector.scalar_tensor_tensor(
                        hnext[:].rearrange("p b d -> p (b d)"),
                        ex[:], -1.0, relu[:], op0=OP.add, op1=OP.add)
                    nc.sync.dma_start(
                        h_of[L + 1][g * SLICE:(g + 1) * SLICE, :].rearrange(
                            "(b p) d -> p b d", p=P),
                        hnext[:])
                    emit_transform(L + 1, g, hnext)
                else:
                    o32 = epip.tile([P, GS, F], fp32, tag="o32")
                    avh = av[:].rearrange("p b (h f) -> p b h f", f=F)
                    nc.vector.tensor_tensor(out=o32[:], in0=avh[:, :, 0, :],
                                            in1=avh[:, :, 1, :], op=OP.add)
                    nc.vector.tensor_tensor(out=o32[:], in0=o32[:],
                                            in1=avh[:, :, 2, :], op=OP.add)
                    nc.vector.tensor_tensor(out=o32[:], in0=o32[:],
                                            in1=avh[:, :, 3, :], op=OP.add)
                    nc.vector.tensor_scalar_mul(o32[:], o32[:], 0.25)
                    nc.sync.dma_start(
                        out_sh[g * SLICE:(g + 1) * SLICE, :].rearrange(
                            "(b p) f -> p b f", p=P),
                        o32[:])

            # ---- layer-0 prologue: per-group transform + collective
            for g in range(NG):
                hsb = hsbp.tile([P, GS, D], fp16, tag="hsb")
                nc.sync.dma_start(
                    hsb[:],
                    feats[g * SLICE:(g + 1) * SLICE, :].rearrange(
                        "(b p) d -> p b d", p=P))
                emit_transform(0, g, hsb)

            # ---- layers
            for L in range(3):
                for g in range(NG):
                    acc_g = accp.tile([P, GS, 132], fp32, tag="acc")
                    blk_seen = set()
                    for k in range(NR):
                        r = (g + k) % NR
                        pm_tiles = {}
                        for (ncc, col, cs) in group_calls[r][g]:
                            ni = ncc * P
                            nseg = sum(len(x) for x in cs)
                            scol0 = cs[0][0][1]
                            Gt = gp.tile([P, CALL_CH, TW], fp16, tag="G")
                            nc.gpsimd.dma_gather(
                                Gt[:, 0:ncc, :],
                                tables[L][r][:],
                                idxs_res[r][:, col:col + ni // 16],
                                num_idxs=ni, num_idxs_reg=ni,
                                elem_size=TW,
                                queue_num=qctr[0] % QUEUES,
                                single_packet=True)
                            qctr[0] += 1
                            OE = ohp.tile([P, SEGW, P], fp16, tag="OE")
                            nc.vector.tensor_tensor(
                                out=OE[:, 0:nseg, :],
                                in0=iota8[:, 0:nseg, :],
                                in1=dcol_res[r][:, scol0:scol0 + nseg]
                                    .unsqueeze(2).to_broadcast([P, nseg, P]),
                                op=OP.is_equal)
                            erp = psep.tile([P, CALL_CH * 4], fp32,
                                            space="PSUM", tag="er")
                            nc.tensor.matmul(
                                erp[:, 0:ncc * 4].rearrange(
                                    "p (c h) -> p c h", h=4),
                                ident[:], Gt[:, 0:ncc, 128:132],
                                start=True, stop=False)
                            OTs = otp.tile([P, SEGW, P], fp16, tag="OT")
                            for s0 in range(0, nseg, 8):
                                nb = min(8, nseg - s0)
                                pt = pstp.tile([P, 8, P], fp16, space="PSUM",
                                               tag="tp")
                                for j in range(nb):
                                    nc.tensor.transpose(pt[:, j, :],
                                                        OE[:, s0 + j, :],
                                                        ident[:])
                                nc.scalar.activation(OTs[:, s0:s0 + nb, :],
                                                     pt[:, 0:nb, :], AF.Copy)
                            si = 0
                            for c in range(ncc):
                                for t, (b, scol, bst, bsp) in enumerate(cs[c]):
                                    nc.tensor.matmul(
                                        erp[:, c * 4:(c + 1) * 4],
                                        OTs[:, si, :], er_all[L][:, b, :],
                                        start=False,
                                        stop=(t == len(cs[c]) - 1))
                                    si += 1
                            e02 = smp.tile([P, CALL_CH * 4], fp32,
                                           tag="e02")
                            nc.scalar.activation(e02[:, 0:ncc * 4],
                                                 erp[:, 0:ncc * 4],
                                                 AF.Copy, scale=0.2)
                            lr = smp.tile([P, CALL_CH * 4], fp32, tag="lr")
                            nc.vector.tensor_tensor(
                                out=lr[:, 0:ncc * 4], in0=e02[:, 0:ncc * 4],
                                in1=erp[:, 0:ncc * 4], op=OP.max)
                            V = gp.tile([P, CALL_CH, 132], fp16, tag="V")
                            nc.scalar.activation(
                                V[:, 0:ncc, 128:132],
                                lr[:, 0:ncc * 4].rearrange(
                                    "p (c h) -> p c h", h=4),
                                AF.Exp)
                            nc.vector.tensor_tensor(
                                out=V[:, 0:ncc, 0:128].rearrange(
                                    "p c (h f) -> p c h f", f=F),
                                in0=Gt[:, 0:ncc, 0:128].rearrange(
                                    "p c (h f) -> p c h f", f=F),
                                in1=V[:, 0:ncc, 128:132]
                                    .unsqueeze(3).to_broadcast([P, ncc, 4, F]),
                                op=OP.mult)
                            si = 0
                            for c in range(ncc):
                                for (b, scol, bst, bsp) in cs[c]:
                                    if b not in pm_tiles:
                                        pm_tiles[b] = psp.tile(
                                            [P, 136], fp32, space="PSUM",
                                            tag="pm", name=f"pm{L}_{r}_{b}")
                                    nc.tensor.matmul(pm_tiles[b][:, 0:132],
                                                     OE[:, si, :], V[:, c, :],
                                                     start=bst, stop=bsp)
                                    si += 1
                                    if bsp:
                                        if b in blk_seen:
                                            nc.vector.tensor_tensor(
                                                out=acc_g[:, b - g * GS, :],
                                                in0=acc_g[:, b - g * GS, :],
                                                in1=pm_tiles[b][:, 0:132],
                                                op=OP.add)
                                        else:
                                            nc.vector.tensor_copy(
                                                acc_g[:, b - g * GS, :],
                                                pm_tiles[b][:, 0:132])
                                            blk_seen.add(b)
                                        del pm_tiles[b]
                    emit_epilogue(L, g, acc_g)
    nc.compile()
    return nc


def kernel(**inputs):
    feats_f32 = np.asarray(inputs["features"], np.float32)
    src = np.asarray(inputs["src"]).astype(np.int64)
    dst = np.asarray(inputs["dst"]).astype(np.int64)
    n_nodes = feats_f32.shape[0]

    core_nodes, sched, data = _preprocess(src, dst, n_nodes)

    # weights: Wcat[L] = [W | Wl | Wr] with Wl = sum_f W[:,h,f]*al[h,f]
    wcat = np.zeros((3, D, 136), np.float16)
    for L, (wn, an, bn) in enumerate([("W0", "al0", "ar0"),
                                      ("W1", "al1", "ar1"),
                                      ("W2", "al2", "ar2")]):
        W = np.asarray(inputs[wn], np.float32)
        al = np.asarray(inputs[an], np.float32)
        ar = np.asarray(inputs[bn], np.float32)
        Wh = W.reshape(D, HEADS, F)
        wcat[L, :, 0:128] = W.astype(np.float16)
        wcat[L, :, 128:132] = np.einsum("dhf,hf->dh", Wh, al).astype(np.float16)
        wcat[L, :, 132:136] = np.einsum("dhf,hf->dh", Wh, ar).astype(np.float16)

    key = (n_nodes, src.shape[0])
    if key not in _CACHE:
        _CACHE[key] = _build(sched)
    nc = _CACHE[key]

    in_maps = []
    for c in range(NCORES):
        f16 = np.zeros((G * P, D), np.float16)
        nodes_c = core_nodes[c]
        f16[:len(nodes_c)] = feats_f32[nodes_c].astype(np.float16)
        in_maps.append({
            "feats": f16,
            "wcat": wcat,
            "idx_t": data["idx_tiles"][c],
            "dcol_t": data["dcol"][c],
        })

    global LAST_PREP
    if PREPARE_ONLY:
        LAST_PREP = (nc, in_maps, core_nodes)
        return np.zeros((n_nodes, F), np.float32)

    import os
    trace = bool(int(os.environ.get("TRN_KERNEL_TRACE", "0")))
    res = run_bass_kernel_spmd(nc, in_maps, core_ids=list(range(NCORES)),
                               trace=trace)
    global LAST_RESULTS
    LAST_RESULTS = res
    out = np.zeros((n_nodes, F), np.float32)
    for c in range(NCORES):
        nodes_c = core_nodes[c]
        out[nodes_c] = res.results[c]["out_sh"][:len(nodes_c)]
    return out
